# revision 24
# baseline (speedup 1.0000x reference)
"""Trainium2 Bass kernel for the AP-model RHS:
    out = concat(S @ u + 8*u*(1-u)*(u-par) - u*v,  -0.01*(8*u*(u-par-1) + v))
with D=8192, S row-sharded across 8 NeuronCores (1024 rows each).

Strategy (pure SPMD, no device collectives; the 8 KB row-concat happens
host-side):
  - the host hands each core a PACKED TRANSPOSE of its row-shard:
    st[p, jl*1024+m] = S[c*1024+m, (ti*J+jl)*128+p], so every DMA tile is
    a fully contiguous 4 MB block with the contraction dim on partitions
  - the matvec runs on the otherwise-idle TensorEngine as float32r
    matmuls (1 cycle/row): psum[1, m] += u_chunk[128,1].T @ st_tile[128, m]
    accumulated over all 64 k-chunks; the last 4 MB tile streams in 2 MB
    chunks (1 MB x4) so the tail is short; deep buffering (bufs=4) hides
    the per-tile DMA-completion/semaphore latency
  - reaction terms are a handful of [1, 1024] VectorE ops; pde1 adds the
    PSUM accumulator directly
The kernel is HBM-bound: 32 MB of S per core at the DMA streaming rate.
"""

import numpy as np

import concourse.bacc as bacc
import concourse.mybir as mybir
import concourse.tile as tile
from concourse.bass_utils import run_bass_kernel_spmd

D = 8192
N_CORES = 8
ROWS = D // N_CORES          # 1024 rows of S per core
NKC = D // 128               # 64 k-chunks of 128
F32 = mybir.dt.float32
F32R = mybir.dt.float32r
K_PARAM = 8.0
EPS_PARAM = 0.01

J = 8                        # k-chunks per big DMA tile (4 MB)
NBIGT = 7                    # 7 big tiles; the 8th streams as chunks
JC = 2                       # k-chunks per tail chunk (1 MB)
NCH = J // JC                # 2 tail chunks

_CACHE = {}


def _emit_body(nc, big_pool, ch_pool, small_pool, psum_pool,
               st_ext, loc_ext, out_ext, u_sb):
    mult = mybir.AluOpType.mult
    add = mybir.AluOpType.add
    sub = mybir.AluOpType.subtract

    acc = psum_pool.tile([1, ROWS], F32, tag="acc")

    loc_sb = small_pool.tile([1, 3 * ROWS], F32, tag="loc")
    nc.scalar.dma_start(out=loc_sb[:], in_=loc_ext[:])

    def matvec(tile_ap, jl_count, j0):
        # tile_ap: [128, jl_count*1024] slice of the packed transpose;
        # chunk jl holds k = (j0+jl)*128 + p
        for jl in range(jl_count):
            j = j0 + jl
            for h in range(2):
                nc.tensor.matmul(
                    acc[0:1, h * 512:(h + 1) * 512],
                    lhsT=u_sb[:, j:j + 1],
                    rhs=tile_ap[:, jl * ROWS + h * 512: jl * ROWS + (h + 1) * 512],
                    start=(j == 0),
                    stop=(j == NKC - 1),
                )

    for ti in range(NBIGT):
        s_tile = big_pool.tile([128, J * ROWS], F32R, tag="big")
        nc.sync.dma_start(
            out=s_tile[:], in_=st_ext[ti * 128:(ti + 1) * 128, :])
        matvec(s_tile[:], J, ti * J)
    for q in range(NCH):
        ch_tile = ch_pool.tile([128, JC * ROWS], F32R, tag="ch")
        nc.sync.dma_start(
            out=ch_tile[:],
            in_=st_ext[NBIGT * 128:(NBIGT + 1) * 128,
                       q * JC * ROWS:(q + 1) * JC * ROWS])
        matvec(ch_tile[:], JC, NBIGT * J + q * JC)

    # --- reaction terms on [1, 1024] tiles (DVE, overlapped w/ stream)
    u_t = loc_sb[0:1, 0:ROWS]
    v_t = loc_sb[0:1, ROWS:2 * ROWS]
    par_t = loc_sb[0:1, 2 * ROWS:3 * ROWS]
    out_sb = small_pool.tile([1, 2 * ROWS], F32, tag="osb")
    s1 = small_pool.tile([1, ROWS], F32, tag="s1")
    s2 = small_pool.tile([1, ROWS], F32, tag="s2")
    s3 = small_pool.tile([1, ROWS], F32, tag="s3")

    nc.vector.tensor_tensor(out=s1[:], in0=u_t, in1=par_t, op=sub)      # u-par
    nc.vector.tensor_scalar_sub(out=s2[:], in0=s1[:], scalar1=1.0)      # u-par-1
    nc.vector.tensor_tensor(out=s2[:], in0=u_t, in1=s2[:], op=mult)     # u(u-par-1)
    nc.vector.tensor_scalar_mul(out=s2[:], in0=s2[:],
                                scalar1=-K_PARAM * EPS_PARAM)
    nc.vector.tensor_scalar_mul(out=s3[:], in0=v_t, scalar1=EPS_PARAM)  # .01v
    nc.vector.tensor_tensor(out=out_sb[0:1, ROWS:2 * ROWS],
                            in0=s2[:], in1=s3[:], op=sub)               # pde2
    nc.vector.tensor_tensor(out=s2[:], in0=u_t, in1=u_t, op=mult)       # u^2
    nc.vector.tensor_tensor(out=s2[:], in0=u_t, in1=s2[:], op=sub)      # u(1-u)
    nc.vector.tensor_tensor(out=s2[:], in0=s2[:], in1=s1[:], op=mult)
    nc.vector.tensor_tensor(out=s3[:], in0=u_t, in1=v_t, op=mult)       # uv
    # s2 = 8*s2 - s3
    nc.vector.scalar_tensor_tensor(out=s2[:], in0=s2[:], scalar=K_PARAM,
                                   in1=s3[:], op0=mult, op1=sub)
    # pde1 = MK + s2  (reads the PSUM accumulator directly)
    nc.vector.tensor_tensor(out=out_sb[0:1, 0:ROWS], in0=acc[0:1, :],
                            in1=s2[:], op=add)

    nc.sync.dma_start(out=out_ext[:], in_=out_sb[:])


def build_nc(reps=1):
    nc = bacc.Bacc("TRN2", target_bir_lowering=False, debug=False,
                   num_devices=N_CORES)

    # packed transpose of the row-shard (see module docstring / make_in_maps)
    st_ext = nc.dram_tensor("st", [(NBIGT + 1) * 128, J * ROWS], F32R,
                            kind="ExternalInput")
    uc_ext = nc.dram_tensor("uc", [128, NKC], F32R, kind="ExternalInput")
    # loc = [u_c, v_c, par_c] on one row
    loc_ext = nc.dram_tensor("loc", [1, 3 * ROWS], F32, kind="ExternalInput")
    out_ext = nc.dram_tensor("out", [1, 2 * ROWS], F32, kind="ExternalOutput")

    with tile.TileContext(nc) as tc:
        with (
            tc.tile_pool(name="const", bufs=1) as const_pool,
            tc.tile_pool(name="big_pool", bufs=4) as big_pool,
            tc.tile_pool(name="ch_pool", bufs=4) as ch_pool,
            tc.tile_pool(name="small", bufs=1) as small_pool,
            tc.tile_pool(name="psum", bufs=4, space="PSUM") as psum_pool,
        ):
            u_sb = const_pool.tile([128, NKC], F32R)
            # scalar queue: keeps the sync HWDGE ring free so the first
            # S-tile DMA issues immediately
            nc.scalar.dma_start(out=u_sb[:], in_=uc_ext[:])
            for _rep in range(reps):
                _emit_body(nc, big_pool, ch_pool, small_pool, psum_pool,
                           st_ext, loc_ext, out_ext, u_sb)

    nc.compile()
    return nc


def _get_nc():
    if "nc" not in _CACHE:
        _CACHE["nc"] = build_nc()
    return _CACHE["nc"]


def make_in_maps(y, S, par):
    u = y[:D]
    v = y[D:2 * D]
    par_flat = par.reshape(-1)
    uc = np.ascontiguousarray(u.reshape(NKC, 128).T)
    in_maps = []
    for c in range(N_CORES):
        sl = slice(c * ROWS, (c + 1) * ROWS)
        # st[p, jl*1024+m] = S[c*1024+m, (ti*J+jl)*128+p]
        st = np.ascontiguousarray(
            S[sl].T.reshape(NBIGT + 1, J, 128, ROWS)
                   .transpose(0, 2, 1, 3)
                   .reshape((NBIGT + 1) * 128, J * ROWS))
        loc = np.concatenate([u[sl], v[sl], par_flat[sl]]).reshape(1, 3 * ROWS)
        in_maps.append({
            "st": st,
            "uc": uc,
            "loc": np.ascontiguousarray(loc),
        })
    return in_maps


def assemble_output(results):
    full = np.empty(2 * D, np.float32)
    for c in range(N_CORES):
        o = results[c]["out"][0]         # [2048]
        full[c * ROWS:(c + 1) * ROWS] = o[0:ROWS]
        full[D + c * ROWS:D + (c + 1) * ROWS] = o[ROWS:2 * ROWS]
    return full


def kernel(t=None, y=None, S=None, par=None, **_unused):
    y = np.asarray(y, np.float32)
    S = np.asarray(S, np.float32)
    par = np.asarray(par, np.float32)
    nc = _get_nc()
    in_maps = make_in_maps(y, S, par)
    res = run_bass_kernel_spmd(nc, in_maps, core_ids=list(range(N_CORES)))
    return assemble_output(res.results)
